# revision 15
# baseline (speedup 1.0000x reference)
"""Trainium2 Bass kernel for the coverage-attention module.

Self-contained: hardcodes shapes B=128, S=512, D=1024, 8 NeuronCores,
data-parallel over batch (16 rows per core).

Per-core dataflow (all matmuls bf16 operands, fp32 PSUM accumulation):
  att.T[e_tile, s]  = sum_dt Wh.T[dt, e_tile].T @ enc.T[dt, s]   (PE)
                      + wc[e_tile] (x) cov[s]                    (K=1 aug MM)
  tanh via ACT with per-partition bias = dec_feat.T[:, b]        (ACT)
  score[1, s]       = sum_et v[e_tile].T @ tanh[e_tile, s]       (PE, M=1)
  softmax on a single partition row; the reference's
  softmax->mask->renormalize collapses to e*mask/sum(e*mask)     (DVE)
  aw broadcast to 128 partitions via ones-outer-product matmul   (PE)
  context.T[dt, b]  = reduce_s(enc.T[dt, s] * aw_bcast)          (DVE)
  context transposed back via PE transpose at the end.

Startup is DMA-starved (~6 MB of weights + first enc tiles), so the
first two batches run att-only with PSUM evicted to SBUF, dec_feat is
computed after them (so Ws.T loads after enc1/enc2), and their tanh /
score / softmax are deferred until dec_feat lands. The last batch uses a
PE outer-product broadcast instead of the DMA broadcast to shorten the
serial tail.
"""

from contextlib import ExitStack

import numpy as np
import ml_dtypes

from concourse import bacc, tile, mybir
from concourse.bass_utils import run_bass_kernel_spmd

B, S, H = 128, 512, 512
D = 2 * H          # 1024
NCORES = 8
BL = B // NCORES   # 16 batch rows per core
NT = D // 128      # 8 tiles of 128 along D

BF = mybir.dt.bfloat16
F32 = mybir.dt.float32
bf16 = ml_dtypes.bfloat16

_CACHE = {}


def _build(repeat=1):
    nc = bacc.Bacc("TRN2", target_bir_lowering=False, debug=False,
                   num_devices=NCORES)

    encT = nc.dram_tensor("encT", [BL, D, S], BF, kind="ExternalInput").ap()
    whT = nc.dram_tensor("whT", [D, D], BF, kind="ExternalInput").ap()
    wsT = nc.dram_tensor("wsT", [D, D], BF, kind="ExternalInput").ap()
    stT = nc.dram_tensor("stT", [128, NT * BL], BF, kind="ExternalInput").ap()
    wsb = nc.dram_tensor("wsb", [128, NT], F32, kind="ExternalInput").ap()
    vT = nc.dram_tensor("vT", [128, NT], BF, kind="ExternalInput").ap()
    wc = nc.dram_tensor("wc", [1, D], BF, kind="ExternalInput").ap()
    covb = nc.dram_tensor("covb", [1, BL * S], BF, kind="ExternalInput").ap()
    covf = nc.dram_tensor("covf", [1, BL * S], F32, kind="ExternalInput").ap()
    maskf = nc.dram_tensor("maskf", [1, BL * S], F32, kind="ExternalInput").ap()
    ident = nc.dram_tensor("ident", [128, 128], F32, kind="ExternalInput").ap()

    ctx_o = nc.dram_tensor("ctx", [BL, D], F32, kind="ExternalOutput").ap()
    aw_o = nc.dram_tensor("aw", [BL, S], F32, kind="ExternalOutput").ap()
    ncov_o = nc.dram_tensor("ncov", [BL, S], F32, kind="ExternalOutput").ap()

    AF = mybir.ActivationFunctionType
    AX = mybir.AxisListType
    ALU = mybir.AluOpType

    with tile.TileContext(nc) as tc, ExitStack() as es:
        cp = es.enter_context(tc.tile_pool(name="const", bufs=1))

        # --- constants, in the order the PE will need them ---
        wc_sb = cp.tile([1, D], BF, tag="wc")
        nc.sync.dma_start(wc_sb[:], wc[:])

        encp = es.enter_context(tc.tile_pool(name="enc", bufs=4))

        covbp = es.enter_context(tc.tile_pool(name="covbp", bufs=4))

        def emit_covrow(b):
            cvb = covbp.tile([1, S], BF, tag="cvb")
            nc.sync.dma_start(cvb[:], covb[0:1, b * S:(b + 1) * S])
            return cvb

        def emit_load(b):
            et_sb = encp.tile([128, NT * S], BF, tag="enc")
            for dt in range(NT):
                nc.sync.dma_start(et_sb[:, dt * S:(dt + 1) * S],
                                  encT[b, dt * 128:(dt + 1) * 128, :])
            return et_sb

        stT_sb = cp.tile([128, NT * BL], BF, tag="stT")
        nc.sync.dma_start(stT_sb[:], stT[:])
        cv0 = emit_covrow(0)
        cv1 = emit_covrow(1)
        cv2 = emit_covrow(2)

        # DMA order = consumption order: Wh.T and enc0 interleaved (b=0 att),
        # then enc1/enc2 prefetch, then Ws.T (dec runs after b1), then the
        # small constants. tanh/score for b0/b1 are deferred until dec is
        # ready, so Ws.T is not needed early.
        wht_sb = []
        enc0_sb = encp.tile([128, NT * S], BF, tag="enc")
        for t in range(NT):
            w1 = cp.tile([128, D], BF, tag=f"wht{t}", name=f"wht{t}")
            nc.sync.dma_start(w1[:], whT[t * 128:(t + 1) * 128, :])
            wht_sb.append(w1)
            nc.sync.dma_start(enc0_sb[:, t * S:(t + 1) * S],
                              encT[0, t * 128:(t + 1) * 128, :])

        enc1_sb = emit_load(1)
        enc2_sb = emit_load(2)
        wst_sb = []
        for t in range(NT):
            w2 = cp.tile([128, D], BF, tag=f"wst{t}", name=f"wst{t}")
            nc.sync.dma_start(w2[:], wsT[t * 128:(t + 1) * 128, :])
            wst_sb.append(w2)
        wsb_sb = cp.tile([128, NT], F32, tag="wsb")
        nc.sync.dma_start(wsb_sb[:], wsb[:])
        vT_sb = cp.tile([128, NT], BF, tag="vT")
        nc.sync.dma_start(vT_sb[:], vT[:])
        id_sb = cp.tile([128, 128], F32, tag="ident")
        nc.sync.dma_start(id_sb[:], ident[:])
        ones_sb = cp.tile([1, 128], BF, tag="ones")
        nc.vector.memset(ones_sb[:], 1.0)

        dec_sb = [cp.tile([128, BL], F32, tag=f"dec{t}", name=f"dec{t}")
                  for t in range(NT)]
        ctx_cols = [cp.tile([128, BL], F32, tag=f"ctxc{t}", name=f"ctxc{t}")
                    for t in range(NT)]
        ctx_out = cp.tile([BL, D], F32, tag="ctxout")

        tanhp = es.enter_context(tc.tile_pool(name="tanh", bufs=3))
        b0attp = es.enter_context(tc.tile_pool(name="b0att", bufs=NT))
        decpsp = es.enter_context(tc.tile_pool(name="decps", bufs=1, space="PSUM"))
        attp = es.enter_context(tc.tile_pool(name="attps", bufs=4, space="PSUM"))
        scorep = es.enter_context(tc.tile_pool(name="scoreps", bufs=2, space="PSUM"))
        trp = es.enter_context(tc.tile_pool(name="trps", bufs=1, space="PSUM"))
        smp = es.enter_context(tc.tile_pool(name="sm", bufs=2))
        awbcp = es.enter_context(tc.tile_pool(name="awbc", bufs=2))
        awdp = es.enter_context(tc.tile_pool(name="awd", bufs=2, space="DRAM"))
        tmpp = es.enter_context(tc.tile_pool(name="ctmp", bufs=2))

        def emit_att_only(b, et_sb, cvb):
            # att groups with PSUM evicted to SBUF; tanh/score deferred
            # (used for b=0/1 while dec_feat is not yet available)
            att_sb = []
            for et in range(NT):
                ap_ = attp.tile([128, S], F32, tag="att")
                nc.tensor.matmul(
                    ap_[:],
                    wc_sb[0:1, et * 128:(et + 1) * 128],
                    cvb[0:1, :],
                    start=True, stop=False,
                )
                for dt in range(NT):
                    nc.tensor.matmul(
                        ap_[:],
                        wht_sb[dt][:, et * 128:(et + 1) * 128],
                        et_sb[:, dt * S:(dt + 1) * S],
                        start=False, stop=(dt == NT - 1),
                    )
                sb = b0attp.tile([128, S], F32, tag=f"b0att{b}",
                                 name=f"b0att{b}_{et}")
                nc.scalar.copy(sb[:], ap_[:])
                att_sb.append(sb)
            return att_sb

        def emit_dec():
            # dec_feat.T columns (et, b) share ONE PSUM bank; only the very
            # first MM carries start=True — later first-writes to untouched
            # bytes reset via the per-element has_written semantics.
            dp = decpsp.tile([128, NT * BL], F32, tag="decps")
            for dt in range(NT):
                for et in range(NT):
                    nc.tensor.matmul(
                        dp[:, et * BL:(et + 1) * BL],
                        wst_sb[dt][:, et * 128:(et + 1) * 128],
                        stT_sb[:, dt * BL:(dt + 1) * BL],
                        start=(dt == 0 and et == 0), stop=(dt == NT - 1),
                        skip_group_check=True,
                    )
            for et in range(NT):
                nc.scalar.activation(dec_sb[et][:],
                                     dp[:, et * BL:(et + 1) * BL],
                                     AF.Identity, bias=wsb_sb[:, et:et + 1])

        def emit_deferred_score(b, att_sb):
            sc = scorep.tile([1, S], F32, tag="score")
            prev = None
            for et in range(NT):
                th = tanhp.tile([128, S], BF, tag="tanh")
                nc.scalar.activation(th[:], att_sb[et][:], AF.Tanh,
                                     bias=dec_sb[et][:, b:b + 1])
                if prev is not None:
                    pet, pth = prev
                    nc.tensor.matmul(sc[:], vT_sb[:, pet:pet + 1], pth[:],
                                     start=(pet == 0), stop=False)
                prev = (et, th)
            pet, pth = prev
            nc.tensor.matmul(sc[:], vT_sb[:, pet:pet + 1], pth[:],
                             start=False, stop=True)
            return sc

        def emit_att_score(b, et_sb, cvb):
            sc = scorep.tile([1, S], F32, tag="score")
            prev = None
            for et in range(NT):
                ap_ = attp.tile([128, S], F32, tag="att")
                nc.tensor.matmul(
                    ap_[:],
                    wc_sb[0:1, et * 128:(et + 1) * 128],
                    cvb[0:1, :],
                    start=True, stop=False,
                )
                for dt in range(NT):
                    nc.tensor.matmul(
                        ap_[:],
                        wht_sb[dt][:, et * 128:(et + 1) * 128],
                        et_sb[:, dt * S:(dt + 1) * S],
                        start=False, stop=(dt == NT - 1),
                    )
                th = tanhp.tile([128, S], BF, tag="tanh")
                nc.scalar.activation(th[:], ap_[:], AF.Tanh,
                                     bias=dec_sb[et][:, b:b + 1])
                if prev is not None:
                    pet, pth = prev
                    nc.tensor.matmul(sc[:], vT_sb[:, pet:pet + 1], pth[:],
                                     start=(pet == 0), stop=False)
                prev = (et, th)
            pet, pth = prev
            nc.tensor.matmul(sc[:], vT_sb[:, pet:pet + 1], pth[:],
                             start=False, stop=True)
            return sc

        def emit_post(b, et_sb, sc, last=False):
            exp_sb = smp.tile([1, S], F32, tag="exp")
            nc.scalar.activation(exp_sb[:], sc[:], AF.Exp)
            mrow = smp.tile([1, S], F32, tag="mrow")
            nc.sync.dma_start(mrow[:], maskf[0:1, b * S:(b + 1) * S])
            em = smp.tile([1, S], F32, tag="emask")
            nc.vector.tensor_mul(em[:], exp_sb[:], mrow[:])
            dn = smp.tile([1, 1], F32, tag="dn")
            nc.vector.reduce_sum(dn[:], em[:], axis=AX.X)
            rd = smp.tile([1, 1], F32, tag="rd")
            nc.vector.reciprocal(rd[:], dn[:])
            awr = smp.tile([1, S], F32, tag="awr")
            nc.vector.tensor_scalar_mul(awr[:], em[:], rd[:])
            cfrow = smp.tile([1, S], F32, tag="cfrow")
            nc.sync.dma_start(cfrow[:], covf[0:1, b * S:(b + 1) * S])
            ncv = smp.tile([1, S], F32, tag="ncv")
            nc.vector.tensor_add(ncv[:], awr[:], cfrow[:])
            nc.sync.dma_start(aw_o[b:b + 1, :], awr[:])
            nc.sync.dma_start(ncov_o[b:b + 1, :], ncv[:])
            awb = smp.tile([1, S], BF, tag="awb")
            nc.vector.tensor_copy(awb[:], awr[:])
            awbc = awbcp.tile([128, S], BF, tag="awbc")
            if last:
                # final batch: PE outer-product broadcast avoids two DMA
                # latencies on the kernel's serial tail
                bc = attp.tile([128, S], F32, tag="att", name="bclast")
                nc.tensor.matmul(bc[:], ones_sb[:], awb[:], start=True,
                                 stop=True)
                nc.scalar.copy(awbc[:], bc[:])
            else:
                awd = awdp.tile([1, S], BF, tag="awd")
                nc.sync.dma_start(awd[:], awb[:])
                nc.sync.dma_start(awbc[:], awd[:].to_broadcast([128, S]))
            # note: tensor_tensor_reduce wedges the device here (HW-only
            # failure; CoreSim accepts it) — use separate mul + reduce.
            for dt in range(NT):
                tmp = tmpp.tile([128, S], BF, tag="tmp")
                nc.vector.tensor_mul(tmp[:], et_sb[:, dt * S:(dt + 1) * S],
                                     awbc[:])
                nc.vector.reduce_sum(ctx_cols[dt][:, b:b + 1], tmp[:],
                                     axis=AX.X)

        for _r in range(repeat):
            if _r == 0:
                att0 = emit_att_only(0, enc0_sb, cv0)
                att1 = emit_att_only(1, enc1_sb, cv1)
                emit_dec()
                sc2 = emit_att_score(2, enc2_sb, cv2)
                sc0 = emit_deferred_score(0, att0)
                sc1 = emit_deferred_score(1, att1)
                et3_sb = emit_load(3)
                sc3 = emit_att_score(3, et3_sb, emit_covrow(3))
                emit_post(0, enc0_sb, sc0)
                emit_post(1, enc1_sb, sc1)
                prev2 = (2, enc2_sb, sc2)
                prev_state = (3, et3_sb, sc3)
                start_b = 4
            else:
                prev2 = None
                prev_state = None
                start_b = 0
            for b in range(start_b, BL):
                et_sb = emit_load(b)
                sc = emit_att_score(b, et_sb, emit_covrow(b))
                if prev2 is not None:
                    emit_post(*prev2)
                    prev2 = None
                if prev_state is not None:
                    emit_post(*prev_state)
                prev_state = (b, et_sb, sc)
            emit_post(*prev_state, last=True)

            for dt in range(NT):
                tr = trp.tile([BL, 128], F32, tag="tr")
                nc.tensor.transpose(tr[:], ctx_cols[dt][:], id_sb[:])
                nc.scalar.copy(ctx_out[:, dt * 128:(dt + 1) * 128], tr[:])
            nc.sync.dma_start(ctx_o[:, :], ctx_out[:])

    nc.compile()
    return nc


def get_nc(repeat=1):
    key = ("nc", repeat)
    if key not in _CACHE:
        _CACHE[key] = _build(repeat)
    return _CACHE[key]


def make_in_maps(h_dec, c_dec, encoder_output, x_padding_masks, coverage_vector,
                 Wh_w, Ws_w, Ws_b, v_w, wc_w):
    h_dec = np.asarray(h_dec, np.float32)
    c_dec = np.asarray(c_dec, np.float32)
    encoder_output = np.asarray(encoder_output, np.float32)
    x_padding_masks = np.asarray(x_padding_masks, np.float32)
    coverage_vector = np.asarray(coverage_vector, np.float32)
    Wh_w = np.asarray(Wh_w, np.float32)
    Ws_w = np.asarray(Ws_w, np.float32)
    Ws_b = np.asarray(Ws_b, np.float32)
    v_w = np.asarray(v_w, np.float32)
    wc_w = np.asarray(wc_w, np.float32)

    whT = np.ascontiguousarray(Wh_w.T).astype(bf16)
    wsT = np.ascontiguousarray(Ws_w.T).astype(bf16)
    wsb = np.ascontiguousarray(Ws_b.reshape(NT, 128).T)
    vT = np.ascontiguousarray(v_w.reshape(NT, 128).T).astype(bf16)
    wc = wc_w.reshape(1, D).astype(bf16)
    ident = np.eye(128, dtype=np.float32)

    s_t = np.concatenate([h_dec[0], c_dec[0]], axis=1)  # (B, D)

    in_maps = []
    for c in range(NCORES):
        sl = slice(c * BL, (c + 1) * BL)
        encTc = np.ascontiguousarray(
            encoder_output[sl].transpose(0, 2, 1)).astype(bf16)
        stTc = np.ascontiguousarray(
            s_t[sl].T.reshape(NT, 128, BL).transpose(1, 0, 2).reshape(128, NT * BL)
        ).astype(bf16)
        covfc = coverage_vector[sl].reshape(1, BL * S)
        in_maps.append({
            "encT": encTc,
            "whT": whT,
            "wsT": wsT,
            "stT": stTc,
            "wsb": wsb,
            "vT": vT,
            "wc": wc,
            "covb": covfc.astype(bf16),
            "covf": np.ascontiguousarray(covfc),
            "maskf": np.ascontiguousarray(
                x_padding_masks[sl].reshape(1, BL * S)),
            "ident": ident,
        })
    return in_maps


def kernel(**inputs):
    nc = get_nc()
    in_maps = make_in_maps(**inputs)
    # the axon-tunneled device occasionally reports a transient
    # NRT_EXEC_UNIT_UNRECOVERABLE / INTERNAL error on the first-ever
    # execution and recovers on retry — retry a few times before giving up
    last_err = None
    for attempt in range(4):
        try:
            res = run_bass_kernel_spmd(nc, in_maps,
                                       core_ids=list(range(NCORES)),
                                       trace=False).results
            break
        except Exception as e:  # noqa: BLE001
            last_err = e
            import time as _time
            _time.sleep(2.0)
    else:
        raise last_err
    ctx = np.concatenate([res[c]["ctx"] for c in range(NCORES)], axis=0)
    aw = np.concatenate([res[c]["aw"] for c in range(NCORES)], axis=0)
    ncov = np.concatenate([res[c]["ncov"] for c in range(NCORES)], axis=0)
    return ctx, aw, ncov


# revision 23
# speedup vs baseline: 1.0140x; 1.0140x over previous
"""Trainium2 Bass kernel for the coverage-attention module.

Self-contained: hardcodes shapes B=128, S=512, D=1024, 8 NeuronCores,
data-parallel over batch (16 rows per core).

Per-core dataflow (all matmuls bf16 operands, fp32 PSUM accumulation):
  att.T[e_tile, s]  = sum_dt Wh.T[dt, e_tile].T @ enc.T[dt, s]   (PE)
                      + wc[e_tile] (x) cov[s]                    (K=1 aug MM)
  tanh via ACT with per-partition bias = dec_feat.T[:, b]        (ACT)
  score[1, s]       = sum_et v[e_tile].T @ tanh[e_tile, s]       (PE, M=1)
  softmax on a single partition row; the reference's
  softmax->mask->renormalize collapses to e*mask/sum(e*mask)     (DVE)
  aw broadcast to 128 partitions via ones-outer-product matmul   (PE)
  context.T[dt, b]  = reduce_s(enc.T[dt, s] * aw_bcast)          (DVE)
  context transposed back via PE transpose at the end.

Startup is DMA-starved (~6 MB of weights + first enc tiles), so the
first two batches run att-only with PSUM evicted to SBUF, dec_feat is
computed after them (so Ws.T loads after enc1/enc2), and their tanh /
score / softmax are deferred until dec_feat lands. The last batch uses a
PE outer-product broadcast instead of the DMA broadcast to shorten the
serial tail.
"""

from contextlib import ExitStack

import numpy as np
import ml_dtypes

from concourse import bacc, tile, mybir
from concourse.bass_utils import run_bass_kernel_spmd

B, S, H = 128, 512, 512
D = 2 * H          # 1024
NCORES = 8
BL = B // NCORES   # 16 batch rows per core
NT = D // 128      # 8 tiles of 128 along D

BF = mybir.dt.bfloat16
F32 = mybir.dt.float32
bf16 = ml_dtypes.bfloat16

_CACHE = {}


def _build(repeat=1):
    nc = bacc.Bacc("TRN2", target_bir_lowering=False, debug=False,
                   num_devices=NCORES)

    encT = nc.dram_tensor("encT", [BL, D, S], BF, kind="ExternalInput").ap()
    whT = nc.dram_tensor("whT", [D, D], BF, kind="ExternalInput").ap()
    wsT = nc.dram_tensor("wsT", [D, D], BF, kind="ExternalInput").ap()
    stT = nc.dram_tensor("stT", [128, NT * BL], BF, kind="ExternalInput").ap()
    wsb = nc.dram_tensor("wsb", [128, NT], F32, kind="ExternalInput").ap()
    vT = nc.dram_tensor("vT", [128, NT], F32, kind="ExternalInput").ap()
    wc = nc.dram_tensor("wc", [1, D], BF, kind="ExternalInput").ap()
    covb = nc.dram_tensor("covb", [1, BL * S], BF, kind="ExternalInput").ap()
    covf = nc.dram_tensor("covf", [1, BL * S], F32, kind="ExternalInput").ap()
    maskf = nc.dram_tensor("maskf", [1, BL * S], F32, kind="ExternalInput").ap()
    ident = nc.dram_tensor("ident", [128, 128], F32, kind="ExternalInput").ap()

    ctx_o = nc.dram_tensor("ctx", [BL, D], F32, kind="ExternalOutput").ap()
    aw_o = nc.dram_tensor("aw", [BL, S], F32, kind="ExternalOutput").ap()
    ncov_o = nc.dram_tensor("ncov", [BL, S], F32, kind="ExternalOutput").ap()

    AF = mybir.ActivationFunctionType
    AX = mybir.AxisListType
    ALU = mybir.AluOpType

    with tile.TileContext(nc) as tc, ExitStack() as es:
        cp = es.enter_context(tc.tile_pool(name="const", bufs=1))

        # --- constants, in the order the PE will need them ---
        wc_sb = cp.tile([1, D], BF, tag="wc")
        nc.sync.dma_start(wc_sb[:], wc[:])

        encp = es.enter_context(tc.tile_pool(name="enc", bufs=4))

        covbp = es.enter_context(tc.tile_pool(name="covbp", bufs=4))

        def emit_covrow(b):
            cvb = covbp.tile([1, S], BF, tag="cvb")
            nc.sync.dma_start(cvb[:], covb[0:1, b * S:(b + 1) * S])
            return cvb

        def emit_load(b):
            et_sb = encp.tile([128, NT * S], BF, tag="enc")
            for dt in range(NT):
                nc.sync.dma_start(et_sb[:, dt * S:(dt + 1) * S],
                                  encT[b, dt * 128:(dt + 1) * 128, :])
            return et_sb

        stT_sb = cp.tile([128, NT * BL], BF, tag="stT")
        nc.sync.dma_start(stT_sb[:], stT[:])
        cv0 = emit_covrow(0)
        cv1 = emit_covrow(1)
        cv2 = emit_covrow(2)

        # DMA order = consumption order: Wh.T and enc0 interleaved (b=0 att),
        # then enc1/enc2 prefetch, then Ws.T (dec runs after b1), then the
        # small constants. tanh/score for b0/b1 are deferred until dec is
        # ready, so Ws.T is not needed early.
        wht_sb = []
        enc0_sb = encp.tile([128, NT * S], BF, tag="enc")
        for t in range(NT):
            w1 = cp.tile([128, D], BF, tag=f"wht{t}", name=f"wht{t}")
            nc.sync.dma_start(w1[:], whT[t * 128:(t + 1) * 128, :])
            wht_sb.append(w1)
            nc.sync.dma_start(enc0_sb[:, t * S:(t + 1) * S],
                              encT[0, t * 128:(t + 1) * 128, :])

        enc1_sb = emit_load(1)
        enc2_sb = emit_load(2)
        wst_sb = []
        for t in range(NT):
            w2 = cp.tile([128, D], BF, tag=f"wst{t}", name=f"wst{t}")
            nc.sync.dma_start(w2[:], wsT[t * 128:(t + 1) * 128, :])
            wst_sb.append(w2)
        wsb_sb = cp.tile([128, NT], F32, tag="wsb")
        nc.sync.dma_start(wsb_sb[:], wsb[:])
        vT_sb = cp.tile([128, NT], F32, tag="vT")
        nc.sync.dma_start(vT_sb[:], vT[:])
        vTb_sb = cp.tile([128, NT], BF, tag="vTb")
        nc.vector.tensor_copy(vTb_sb[:], vT_sb[:])
        id_sb = cp.tile([128, 128], F32, tag="ident")
        nc.sync.dma_start(id_sb[:], ident[:])
        ones_sb = cp.tile([1, 128], BF, tag="ones")
        nc.vector.memset(ones_sb[:], 1.0)
        onec_sb = cp.tile([128, 1], BF, tag="onec")
        nc.vector.memset(onec_sb[:], 1.0)

        dec_sb = [cp.tile([128, BL], F32, tag=f"dec{t}", name=f"dec{t}")
                  for t in range(NT)]
        ctx_cols = [cp.tile([128, BL], F32, tag=f"ctxc{t}", name=f"ctxc{t}")
                    for t in range(NT)]
        ctx_out = cp.tile([BL, D], F32, tag="ctxout")

        tanhp = es.enter_context(tc.tile_pool(name="tanh", bufs=12))
        vsump = es.enter_context(tc.tile_pool(name="vsum", bufs=12))
        b0attp = es.enter_context(tc.tile_pool(name="b0att", bufs=NT))
        decpsp = es.enter_context(tc.tile_pool(name="decps", bufs=1, space="PSUM"))
        attp = es.enter_context(tc.tile_pool(name="attps", bufs=4, space="PSUM"))
        scorep = es.enter_context(tc.tile_pool(name="scoreps", bufs=2, space="PSUM"))
        trp = es.enter_context(tc.tile_pool(name="trps", bufs=1, space="PSUM"))
        smp = es.enter_context(tc.tile_pool(name="sm", bufs=2))
        awbcp = es.enter_context(tc.tile_pool(name="awbc", bufs=2))
        awdp = es.enter_context(tc.tile_pool(name="awd", bufs=2, space="DRAM"))
        tmpp = es.enter_context(tc.tile_pool(name="ctmp", bufs=2))

        def emit_att_only(b, et_sb, cvb):
            # att groups with PSUM evicted to SBUF; tanh/score deferred
            # (used for b=0/1 while dec_feat is not yet available)
            att_sb = []
            for et in range(NT):
                ap_ = attp.tile([128, S], F32, tag="att")
                nc.tensor.matmul(
                    ap_[:],
                    wc_sb[0:1, et * 128:(et + 1) * 128],
                    cvb[0:1, :],
                    start=True, stop=False,
                )
                for dt in range(NT):
                    nc.tensor.matmul(
                        ap_[:],
                        wht_sb[dt][:, et * 128:(et + 1) * 128],
                        et_sb[:, dt * S:(dt + 1) * S],
                        start=False, stop=(dt == NT - 1),
                    )
                sb = b0attp.tile([128, S], F32, tag=f"b0att{b}",
                                 name=f"b0att{b}_{et}")
                nc.scalar.copy(sb[:], ap_[:])
                att_sb.append(sb)
            return att_sb

        def emit_dec():
            # dec_feat.T columns (et, b) share ONE PSUM bank; only the very
            # first MM carries start=True — later first-writes to untouched
            # bytes reset via the per-element has_written semantics.
            dp = decpsp.tile([128, NT * BL], F32, tag="decps")
            for dt in range(NT):
                for et in range(NT):
                    nc.tensor.matmul(
                        dp[:, et * BL:(et + 1) * BL],
                        wst_sb[dt][:, et * 128:(et + 1) * 128],
                        stT_sb[:, dt * BL:(dt + 1) * BL],
                        start=(dt == 0 and et == 0), stop=(dt == NT - 1),
                        skip_group_check=True,
                    )
            for et in range(NT):
                nc.scalar.activation(dec_sb[et][:],
                                     dp[:, et * BL:(et + 1) * BL],
                                     AF.Identity, bias=wsb_sb[:, et:et + 1])

        def emit_vscore_step(et, b, tanh_src, chain):
            th = tanhp.tile([128, S], BF, tag="tanh")
            nc.scalar.activation(th[:], tanh_src[:], AF.Tanh,
                                 bias=dec_sb[et][:, b:b + 1])
            nxt = vsump.tile([128, S], BF, tag="vsum")
            if chain is None:
                nc.vector.tensor_scalar_mul(nxt[:], th[:],
                                            vT_sb[:, et:et + 1])
            else:
                # fused (th * v) + chain in one DVE pass
                nc.vector.scalar_tensor_tensor(
                    nxt[:], th[:], vT_sb[:, et:et + 1], chain[:],
                    op0=ALU.mult, op1=ALU.add)
            return nxt

        def emit_deferred_score(b, att_sb):
            chain = None
            for et in range(NT):
                chain = emit_vscore_step(et, b, att_sb[et], chain)
            return chain

        def emit_att_score(b, et_sb, cvb, pe_score=False):
            chain = None
            sc = (scorep.tile([1, S], F32, tag="score", name="scpe")
                  if pe_score else None)
            prev = None
            for et in range(NT):
                ap_ = attp.tile([128, S], F32, tag="att")
                nc.tensor.matmul(
                    ap_[:],
                    wc_sb[0:1, et * 128:(et + 1) * 128],
                    cvb[0:1, :],
                    start=True, stop=False,
                )
                for dt in range(NT):
                    nc.tensor.matmul(
                        ap_[:],
                        wht_sb[dt][:, et * 128:(et + 1) * 128],
                        et_sb[:, dt * S:(dt + 1) * S],
                        start=False, stop=(dt == NT - 1),
                    )
                if pe_score:
                    th = tanhp.tile([128, S], BF, tag="tanh")
                    nc.scalar.activation(th[:], ap_[:], AF.Tanh,
                                         bias=dec_sb[et][:, b:b + 1])
                    if prev is not None:
                        pet, pth = prev
                        nc.tensor.matmul(sc[:], vTb_sb[:, pet:pet + 1],
                                         pth[:], start=(pet == 0), stop=False)
                    prev = (et, th)
                else:
                    chain = emit_vscore_step(et, b, ap_, chain)
            if pe_score:
                pet, pth = prev
                nc.tensor.matmul(sc[:], vTb_sb[:, pet:pet + 1], pth[:],
                                 start=False, stop=True)
                return sc
            return chain

        def emit_post(b, et_sb, vsum, last=False):
            from concourse.bass import MemorySpace
            if vsum.space == MemorySpace.PSUM:
                sc = vsum  # PE-score path already produced score in PSUM
            else:
                # score[1, s] = partition-sum of vsum via a single
                # ones-column matmul (M=1, one N=512 pass per batch)
                sc = scorep.tile([1, S], F32, tag="score")
                nc.tensor.matmul(sc[:], onec_sb[:], vsum[:], start=True,
                                 stop=True)
            exp_sb = smp.tile([1, S], F32, tag="exp")
            nc.scalar.activation(exp_sb[:], sc[:], AF.Exp)
            mrow = smp.tile([1, S], F32, tag="mrow")
            nc.sync.dma_start(mrow[:], maskf[0:1, b * S:(b + 1) * S])
            em = smp.tile([1, S], F32, tag="emask")
            nc.vector.tensor_mul(em[:], exp_sb[:], mrow[:])
            dn = smp.tile([1, 1], F32, tag="dn")
            nc.vector.reduce_sum(dn[:], em[:], axis=AX.X)
            rd = smp.tile([1, 1], F32, tag="rd")
            nc.vector.reciprocal(rd[:], dn[:])
            awr = smp.tile([1, S], F32, tag="awr")
            nc.vector.tensor_scalar_mul(awr[:], em[:], rd[:])
            cfrow = smp.tile([1, S], F32, tag="cfrow")
            nc.sync.dma_start(cfrow[:], covf[0:1, b * S:(b + 1) * S])
            ncv = smp.tile([1, S], F32, tag="ncv")
            nc.vector.tensor_add(ncv[:], awr[:], cfrow[:])
            nc.sync.dma_start(aw_o[b:b + 1, :], awr[:])
            nc.sync.dma_start(ncov_o[b:b + 1, :], ncv[:])
            awb = smp.tile([1, S], BF, tag="awb")
            nc.vector.tensor_copy(awb[:], awr[:])
            awbc = awbcp.tile([128, S], BF, tag="awbc")
            if last:
                # final batch: PE outer-product broadcast avoids two DMA
                # latencies on the kernel's serial tail
                bc = attp.tile([128, S], F32, tag="att", name="bclast")
                nc.tensor.matmul(bc[:], ones_sb[:], awb[:], start=True,
                                 stop=True)
                nc.scalar.copy(awbc[:], bc[:])
            else:
                awd = awdp.tile([1, S], BF, tag="awd")
                nc.sync.dma_start(awd[:], awb[:])
                nc.sync.dma_start(awbc[:], awd[:].to_broadcast([128, S]))
            # note: tensor_tensor_reduce wedges the device here (HW-only
            # failure; CoreSim accepts it) — use separate mul + reduce.
            for dt in range(NT):
                tmp = tmpp.tile([128, S], BF, tag="tmp")
                nc.vector.tensor_mul(tmp[:], et_sb[:, dt * S:(dt + 1) * S],
                                     awbc[:])
                nc.vector.reduce_sum(ctx_cols[dt][:, b:b + 1], tmp[:],
                                     axis=AX.X)

        for _r in range(repeat):
            if _r == 0:
                att0 = emit_att_only(0, enc0_sb, cv0)
                att1 = emit_att_only(1, enc1_sb, cv1)
                emit_dec()
                sc2 = emit_att_score(2, enc2_sb, cv2)
                sc0 = emit_deferred_score(0, att0)
                sc1 = emit_deferred_score(1, att1)
                et3_sb = emit_load(3)
                sc3 = emit_att_score(3, et3_sb, emit_covrow(3))
                emit_post(0, enc0_sb, sc0)
                emit_post(1, enc1_sb, sc1)
                prev2 = (2, enc2_sb, sc2)
                prev_state = (3, et3_sb, sc3)
                start_b = 4
            else:
                prev2 = None
                prev_state = None
                start_b = 0
            for b in range(start_b, BL):
                et_sb = emit_load(b)
                sc = emit_att_score(b, et_sb, emit_covrow(b),
                                    pe_score=(b >= BL - 2))
                if prev2 is not None:
                    emit_post(*prev2)
                    prev2 = None
                if prev_state is not None:
                    emit_post(*prev_state)
                prev_state = (b, et_sb, sc)
            emit_post(*prev_state, last=True)

            for dt in range(NT):
                tr = trp.tile([BL, 128], F32, tag="tr")
                nc.tensor.transpose(tr[:], ctx_cols[dt][:], id_sb[:])
                nc.scalar.copy(ctx_out[:, dt * 128:(dt + 1) * 128], tr[:])
            nc.sync.dma_start(ctx_o[:, :], ctx_out[:])

    nc.compile()
    return nc


def get_nc(repeat=1):
    key = ("nc", repeat)
    if key not in _CACHE:
        _CACHE[key] = _build(repeat)
    return _CACHE[key]


def make_in_maps(h_dec, c_dec, encoder_output, x_padding_masks, coverage_vector,
                 Wh_w, Ws_w, Ws_b, v_w, wc_w):
    h_dec = np.asarray(h_dec, np.float32)
    c_dec = np.asarray(c_dec, np.float32)
    encoder_output = np.asarray(encoder_output, np.float32)
    x_padding_masks = np.asarray(x_padding_masks, np.float32)
    coverage_vector = np.asarray(coverage_vector, np.float32)
    Wh_w = np.asarray(Wh_w, np.float32)
    Ws_w = np.asarray(Ws_w, np.float32)
    Ws_b = np.asarray(Ws_b, np.float32)
    v_w = np.asarray(v_w, np.float32)
    wc_w = np.asarray(wc_w, np.float32)

    whT = np.ascontiguousarray(Wh_w.T).astype(bf16)
    wsT = np.ascontiguousarray(Ws_w.T).astype(bf16)
    wsb = np.ascontiguousarray(Ws_b.reshape(NT, 128).T)
    vT = np.ascontiguousarray(v_w.reshape(NT, 128).T)
    wc = wc_w.reshape(1, D).astype(bf16)
    ident = np.eye(128, dtype=np.float32)

    s_t = np.concatenate([h_dec[0], c_dec[0]], axis=1)  # (B, D)

    in_maps = []
    for c in range(NCORES):
        sl = slice(c * BL, (c + 1) * BL)
        encTc = np.ascontiguousarray(
            encoder_output[sl].transpose(0, 2, 1)).astype(bf16)
        stTc = np.ascontiguousarray(
            s_t[sl].T.reshape(NT, 128, BL).transpose(1, 0, 2).reshape(128, NT * BL)
        ).astype(bf16)
        covfc = coverage_vector[sl].reshape(1, BL * S)
        in_maps.append({
            "encT": encTc,
            "whT": whT,
            "wsT": wsT,
            "stT": stTc,
            "wsb": wsb,
            "vT": vT,
            "wc": wc,
            "covb": covfc.astype(bf16),
            "covf": np.ascontiguousarray(covfc),
            "maskf": np.ascontiguousarray(
                x_padding_masks[sl].reshape(1, BL * S)),
            "ident": ident,
        })
    return in_maps


def kernel(**inputs):
    nc = get_nc()
    in_maps = make_in_maps(**inputs)
    # the axon-tunneled device occasionally reports a transient
    # NRT_EXEC_UNIT_UNRECOVERABLE / INTERNAL error on the first-ever
    # execution and recovers on retry — retry a few times before giving up
    last_err = None
    for attempt in range(4):
        try:
            res = run_bass_kernel_spmd(nc, in_maps,
                                       core_ids=list(range(NCORES)),
                                       trace=False).results
            break
        except Exception as e:  # noqa: BLE001
            last_err = e
            import time as _time
            _time.sleep(2.0)
    else:
        raise last_err
    ctx = np.concatenate([res[c]["ctx"] for c in range(NCORES)], axis=0)
    aw = np.concatenate([res[c]["aw"] for c in range(NCORES)], axis=0)
    ncov = np.concatenate([res[c]["ncov"] for c in range(NCORES)], axis=0)
    return ctx, aw, ncov


# revision 26
# speedup vs baseline: 1.0976x; 1.0824x over previous
"""Trainium2 Bass kernel for the coverage-attention module.

Self-contained: hardcodes shapes B=128, S=512, D=1024, 8 NeuronCores,
data-parallel over batch (16 rows per core).

Per-core dataflow (all matmuls bf16 operands, fp32 PSUM accumulation):
  att.T[e_tile, s]  = sum_dt Wh.T[dt, e_tile].T @ enc.T[dt, s]   (PE)
                      + wc[e_tile] (x) cov[s]                    (K=1 aug MM)
  tanh via ACT with per-partition bias = dec_feat.T[:, b]        (ACT)
  vsum[e, s]        = sum_et v[e] * tanh[e_tile, s], fused
                      multiply-add chain (scalar_tensor_tensor)  (DVE)
  score[1, s]       = ones.T @ vsum  (ONE M=1 matmul per batch;
                      the last two batches keep per-e_tile PE
                      score matmuls so the serial tail never
                      waits on the DVE chain)                    (PE)
  softmax on a single partition row; the reference's
  softmax->mask->renormalize collapses to e*mask/sum(e*mask)     (DVE)
  aw broadcast to 128 partitions via ones-outer-product matmul   (PE)
  context.T[dt, b]  = reduce_s(enc.T[dt, s] * aw_bcast)          (DVE)
  context transposed back via PE transpose at the end.

Startup is DMA-starved (~6 MB of weights + first enc tiles), so the
first two batches run att-only with PSUM evicted to SBUF, dec_feat is
computed after them (so Ws.T loads after enc1/enc2), and their tanh /
score / softmax are deferred until dec_feat lands. The last batch uses a
PE outer-product broadcast instead of the DMA broadcast to shorten the
serial tail.
"""

from contextlib import ExitStack

import numpy as np
import ml_dtypes

from concourse import bacc, tile, mybir
from concourse.bass_utils import run_bass_kernel_spmd

B, S, H = 128, 512, 512
D = 2 * H          # 1024
NCORES = 8
BL = B // NCORES   # 16 batch rows per core
NT = D // 128      # 8 tiles of 128 along D

BF = mybir.dt.bfloat16
F32 = mybir.dt.float32
bf16 = ml_dtypes.bfloat16

_CACHE = {}


def _build(repeat=1):
    nc = bacc.Bacc("TRN2", target_bir_lowering=False, debug=False,
                   num_devices=NCORES)

    encT = nc.dram_tensor("encT", [BL, D, S], BF, kind="ExternalInput").ap()
    whT = nc.dram_tensor("whT", [D, D], BF, kind="ExternalInput").ap()
    wsT = nc.dram_tensor("wsT", [D, D], BF, kind="ExternalInput").ap()
    stT = nc.dram_tensor("stT", [128, NT * BL], BF, kind="ExternalInput").ap()
    wsb = nc.dram_tensor("wsb", [128, NT], F32, kind="ExternalInput").ap()
    vT = nc.dram_tensor("vT", [128, NT], F32, kind="ExternalInput").ap()
    wc = nc.dram_tensor("wc", [1, D], BF, kind="ExternalInput").ap()
    covb = nc.dram_tensor("covb", [1, BL * S], BF, kind="ExternalInput").ap()
    covf = nc.dram_tensor("covf", [1, BL * S], F32, kind="ExternalInput").ap()
    maskf = nc.dram_tensor("maskf", [1, BL * S], F32, kind="ExternalInput").ap()
    ident = nc.dram_tensor("ident", [128, 128], F32, kind="ExternalInput").ap()

    ctx_o = nc.dram_tensor("ctx", [BL, D], F32, kind="ExternalOutput").ap()
    aw_o = nc.dram_tensor("aw", [BL, S], F32, kind="ExternalOutput").ap()
    ncov_o = nc.dram_tensor("ncov", [BL, S], F32, kind="ExternalOutput").ap()

    AF = mybir.ActivationFunctionType
    AX = mybir.AxisListType
    ALU = mybir.AluOpType

    with tile.TileContext(nc) as tc, ExitStack() as es:
        cp = es.enter_context(tc.tile_pool(name="const", bufs=1))

        # --- constants, in the order the PE will need them ---
        wc_sb = cp.tile([1, D], BF, tag="wc")
        nc.sync.dma_start(wc_sb[:], wc[:])

        encp = es.enter_context(tc.tile_pool(name="enc", bufs=4))

        covbp = es.enter_context(tc.tile_pool(name="covbp", bufs=4))

        def emit_covrow(b):
            cvb = covbp.tile([1, S], BF, tag="cvb")
            nc.sync.dma_start(cvb[:], covb[0:1, b * S:(b + 1) * S])
            return cvb

        def emit_load(b, nq=4):
            # nq transfers per batch: fewer dma_start instructions (the SP
            # sequencer costs ~0.5us issue per DMA) while still spreading
            # across enough HWDGE queues for bandwidth
            et_sb = encp.tile([128, NT * S], BF, tag="enc")
            tpq = NT // nq
            for q in range(nq):
                src3 = encT[b, q * tpq * 128:(q + 1) * tpq * 128, :].rearrange(
                    "(t p) s -> p t s", p=128)
                dst3 = et_sb[:, q * tpq * S:(q + 1) * tpq * S].rearrange(
                    "p (t s) -> p t s", t=tpq)
                nc.sync.dma_start(dst3, src3)
            return et_sb

        stT_sb = cp.tile([128, NT * BL], BF, tag="stT")
        nc.sync.dma_start(stT_sb[:], stT[:])
        cv0 = emit_covrow(0)
        cv1 = emit_covrow(1)
        cv2 = emit_covrow(2)

        # DMA order = consumption order: Wh.T and enc0 interleaved (b=0 att),
        # then enc1/enc2 prefetch, then Ws.T (dec runs after b1), then the
        # small constants. tanh/score for b0/b1 are deferred until dec is
        # ready, so Ws.T is not needed early.
        wht_sb = []
        enc0_sb = encp.tile([128, NT * S], BF, tag="enc")
        for t in range(NT):
            w1 = cp.tile([128, D], BF, tag=f"wht{t}", name=f"wht{t}")
            nc.sync.dma_start(w1[:], whT[t * 128:(t + 1) * 128, :])
            wht_sb.append(w1)
            nc.sync.dma_start(enc0_sb[:, t * S:(t + 1) * S],
                              encT[0, t * 128:(t + 1) * 128, :])

        enc1_sb = emit_load(1)
        enc2_sb = emit_load(2)
        wst_sb = []
        for t in range(NT):
            w2 = cp.tile([128, D], BF, tag=f"wst{t}", name=f"wst{t}")
            nc.sync.dma_start(w2[:], wsT[t * 128:(t + 1) * 128, :])
            wst_sb.append(w2)
        wsb_sb = cp.tile([128, NT], F32, tag="wsb")
        nc.sync.dma_start(wsb_sb[:], wsb[:])
        vT_sb = cp.tile([128, NT], F32, tag="vT")
        nc.sync.dma_start(vT_sb[:], vT[:])
        vTb_sb = cp.tile([128, NT], BF, tag="vTb")
        nc.vector.tensor_copy(vTb_sb[:], vT_sb[:])
        id_sb = cp.tile([128, 128], F32, tag="ident")
        nc.sync.dma_start(id_sb[:], ident[:])
        ones_sb = cp.tile([1, 128], BF, tag="ones")
        nc.vector.memset(ones_sb[:], 1.0)
        onec_sb = cp.tile([128, 1], BF, tag="onec")
        nc.vector.memset(onec_sb[:], 1.0)

        dec_sb = [cp.tile([128, BL], F32, tag=f"dec{t}", name=f"dec{t}")
                  for t in range(NT)]
        ctx_cols = [cp.tile([128, BL], F32, tag=f"ctxc{t}", name=f"ctxc{t}")
                    for t in range(NT)]
        ctx_out = cp.tile([BL, D], F32, tag="ctxout")

        tanhp = es.enter_context(tc.tile_pool(name="tanh", bufs=12))
        vsump = es.enter_context(tc.tile_pool(name="vsum", bufs=12))
        b0attp = es.enter_context(tc.tile_pool(name="b0att", bufs=NT))
        decpsp = es.enter_context(tc.tile_pool(name="decps", bufs=1, space="PSUM"))
        attp = es.enter_context(tc.tile_pool(name="attps", bufs=4, space="PSUM"))
        scorep = es.enter_context(tc.tile_pool(name="scoreps", bufs=2, space="PSUM"))
        trp = es.enter_context(tc.tile_pool(name="trps", bufs=1, space="PSUM"))
        smp = es.enter_context(tc.tile_pool(name="sm", bufs=2))
        awbcp = es.enter_context(tc.tile_pool(name="awbc", bufs=2))
        awdp = es.enter_context(tc.tile_pool(name="awd", bufs=2, space="DRAM"))
        tmpp = es.enter_context(tc.tile_pool(name="ctmp", bufs=2))

        def emit_att_only(b, et_sb, cvb):
            # att groups with PSUM evicted to SBUF; tanh/score deferred
            # (used for b=0/1 while dec_feat is not yet available)
            att_sb = []
            for et in range(NT):
                ap_ = attp.tile([128, S], F32, tag="att")
                nc.tensor.matmul(
                    ap_[:],
                    wc_sb[0:1, et * 128:(et + 1) * 128],
                    cvb[0:1, :],
                    start=True, stop=False,
                )
                for dt in range(NT):
                    nc.tensor.matmul(
                        ap_[:],
                        wht_sb[dt][:, et * 128:(et + 1) * 128],
                        et_sb[:, dt * S:(dt + 1) * S],
                        start=False, stop=(dt == NT - 1),
                    )
                sb = b0attp.tile([128, S], F32, tag=f"b0att{b}",
                                 name=f"b0att{b}_{et}")
                nc.scalar.copy(sb[:], ap_[:])
                att_sb.append(sb)
            return att_sb

        def emit_dec():
            # dec_feat.T columns (et, b) share ONE PSUM bank; only the very
            # first MM carries start=True — later first-writes to untouched
            # bytes reset via the per-element has_written semantics.
            dp = decpsp.tile([128, NT * BL], F32, tag="decps")
            for dt in range(NT):
                for et in range(NT):
                    nc.tensor.matmul(
                        dp[:, et * BL:(et + 1) * BL],
                        wst_sb[dt][:, et * 128:(et + 1) * 128],
                        stT_sb[:, dt * BL:(dt + 1) * BL],
                        start=(dt == 0 and et == 0), stop=(dt == NT - 1),
                        skip_group_check=True,
                    )
            for et in range(NT):
                nc.scalar.activation(dec_sb[et][:],
                                     dp[:, et * BL:(et + 1) * BL],
                                     AF.Identity, bias=wsb_sb[:, et:et + 1])

        def emit_vscore_step(et, b, tanh_src, chain):
            th = tanhp.tile([128, S], BF, tag="tanh")
            nc.scalar.activation(th[:], tanh_src[:], AF.Tanh,
                                 bias=dec_sb[et][:, b:b + 1])
            nxt = vsump.tile([128, S], BF, tag="vsum")
            if chain is None:
                nc.vector.tensor_scalar_mul(nxt[:], th[:],
                                            vT_sb[:, et:et + 1])
            else:
                # fused (th * v) + chain in one DVE pass
                nc.vector.scalar_tensor_tensor(
                    nxt[:], th[:], vT_sb[:, et:et + 1], chain[:],
                    op0=ALU.mult, op1=ALU.add)
            return nxt

        def emit_deferred_score(b, att_sb):
            # b0/b1 scores land in the DVE-heavy prologue region — use PE
            # M=1 score matmuls there instead (the PE has slack while the
            # enc/weight DMAs stream in)
            sc = scorep.tile([1, S], F32, tag="score", name=f"scd{b}")
            prev = None
            for et in range(NT):
                th = tanhp.tile([128, S], BF, tag="tanh")
                nc.scalar.activation(th[:], att_sb[et][:], AF.Tanh,
                                     bias=dec_sb[et][:, b:b + 1])
                if prev is not None:
                    pet, pth = prev
                    nc.tensor.matmul(sc[:], vTb_sb[:, pet:pet + 1], pth[:],
                                     start=(pet == 0), stop=False)
                prev = (et, th)
            pet, pth = prev
            nc.tensor.matmul(sc[:], vTb_sb[:, pet:pet + 1], pth[:],
                             start=False, stop=True)
            return sc

        def emit_att_score(b, et_sb, cvb, pe_score=False):
            chain = None
            sc = (scorep.tile([1, S], F32, tag="score", name="scpe")
                  if pe_score else None)
            prev = None
            for et in range(NT):
                ap_ = attp.tile([128, S], F32, tag="att")
                nc.tensor.matmul(
                    ap_[:],
                    wc_sb[0:1, et * 128:(et + 1) * 128],
                    cvb[0:1, :],
                    start=True, stop=False,
                )
                for dt in range(NT):
                    nc.tensor.matmul(
                        ap_[:],
                        wht_sb[dt][:, et * 128:(et + 1) * 128],
                        et_sb[:, dt * S:(dt + 1) * S],
                        start=False, stop=(dt == NT - 1),
                    )
                if pe_score:
                    th = tanhp.tile([128, S], BF, tag="tanh")
                    nc.scalar.activation(th[:], ap_[:], AF.Tanh,
                                         bias=dec_sb[et][:, b:b + 1])
                    if prev is not None:
                        pet, pth = prev
                        nc.tensor.matmul(sc[:], vTb_sb[:, pet:pet + 1],
                                         pth[:], start=(pet == 0), stop=False)
                    prev = (et, th)
                else:
                    chain = emit_vscore_step(et, b, ap_, chain)
            if pe_score:
                pet, pth = prev
                nc.tensor.matmul(sc[:], vTb_sb[:, pet:pet + 1], pth[:],
                                 start=False, stop=True)
                return sc
            return chain

        def emit_post(b, et_sb, vsum, last=False):
            from concourse.bass import MemorySpace
            if vsum.space == MemorySpace.PSUM:
                sc = vsum  # PE-score path already produced score in PSUM
            else:
                # score[1, s] = partition-sum of vsum via a single
                # ones-column matmul (M=1, one N=512 pass per batch)
                sc = scorep.tile([1, S], F32, tag="score")
                nc.tensor.matmul(sc[:], onec_sb[:], vsum[:], start=True,
                                 stop=True)
            exp_sb = smp.tile([1, S], F32, tag="exp")
            nc.scalar.activation(exp_sb[:], sc[:], AF.Exp)
            mrow = smp.tile([1, S], F32, tag="mrow")
            nc.sync.dma_start(mrow[:], maskf[0:1, b * S:(b + 1) * S])
            em = smp.tile([1, S], F32, tag="emask")
            nc.vector.tensor_mul(em[:], exp_sb[:], mrow[:])
            dn = smp.tile([1, 1], F32, tag="dn")
            nc.vector.reduce_sum(dn[:], em[:], axis=AX.X)
            rd = smp.tile([1, 1], F32, tag="rd")
            nc.vector.reciprocal(rd[:], dn[:])
            awr = smp.tile([1, S], F32, tag="awr")
            nc.vector.tensor_scalar_mul(awr[:], em[:], rd[:])
            cfrow = smp.tile([1, S], F32, tag="cfrow")
            nc.sync.dma_start(cfrow[:], covf[0:1, b * S:(b + 1) * S])
            ncv = smp.tile([1, S], F32, tag="ncv")
            nc.vector.tensor_add(ncv[:], awr[:], cfrow[:])
            nc.sync.dma_start(aw_o[b:b + 1, :], awr[:])
            nc.sync.dma_start(ncov_o[b:b + 1, :], ncv[:])
            awb = smp.tile([1, S], BF, tag="awb")
            nc.vector.tensor_copy(awb[:], awr[:])
            awbc = awbcp.tile([128, S], BF, tag="awbc")
            if last:
                # final batch: PE outer-product broadcast avoids two DMA
                # latencies on the kernel's serial tail
                bc = attp.tile([128, S], F32, tag="att", name="bclast")
                nc.tensor.matmul(bc[:], ones_sb[:], awb[:], start=True,
                                 stop=True)
                nc.scalar.copy(awbc[:], bc[:])
            else:
                awd = awdp.tile([1, S], BF, tag="awd")
                nc.sync.dma_start(awd[:], awb[:])
                nc.sync.dma_start(awbc[:], awd[:].to_broadcast([128, S]))
            # note: tensor_tensor_reduce wedges the device here (HW-only
            # failure; CoreSim accepts it) — use separate mul + reduce.
            for dt in range(NT):
                tmp = tmpp.tile([128, S], BF, tag="tmp")
                nc.vector.tensor_mul(tmp[:], et_sb[:, dt * S:(dt + 1) * S],
                                     awbc[:])
                nc.vector.reduce_sum(ctx_cols[dt][:, b:b + 1], tmp[:],
                                     axis=AX.X)

        for _r in range(repeat):
            if _r == 0:
                att0 = emit_att_only(0, enc0_sb, cv0)
                att1 = emit_att_only(1, enc1_sb, cv1)
                emit_dec()
                sc2 = emit_att_score(2, enc2_sb, cv2)
                sc0 = emit_deferred_score(0, att0)
                sc1 = emit_deferred_score(1, att1)
                et3_sb = emit_load(3)
                sc3 = emit_att_score(3, et3_sb, emit_covrow(3))
                emit_post(0, enc0_sb, sc0)
                emit_post(1, enc1_sb, sc1)
                prev2 = (2, enc2_sb, sc2)
                prev_state = (3, et3_sb, sc3)
                start_b = 4
            else:
                prev2 = None
                prev_state = None
                start_b = 0
            for b in range(start_b, BL):
                et_sb = emit_load(b)
                sc = emit_att_score(b, et_sb, emit_covrow(b),
                                    pe_score=(b >= BL - 2))
                if prev2 is not None:
                    emit_post(*prev2)
                    prev2 = None
                if prev_state is not None:
                    emit_post(*prev_state)
                prev_state = (b, et_sb, sc)
            emit_post(*prev_state, last=True)

            for dt in range(NT):
                tr = trp.tile([BL, 128], F32, tag="tr")
                nc.tensor.transpose(tr[:], ctx_cols[dt][:], id_sb[:])
                nc.scalar.copy(ctx_out[:, dt * 128:(dt + 1) * 128], tr[:])
            nc.sync.dma_start(ctx_o[:, :], ctx_out[:])

    nc.compile()
    return nc


def get_nc(repeat=1):
    key = ("nc", repeat)
    if key not in _CACHE:
        _CACHE[key] = _build(repeat)
    return _CACHE[key]


def make_in_maps(h_dec, c_dec, encoder_output, x_padding_masks, coverage_vector,
                 Wh_w, Ws_w, Ws_b, v_w, wc_w):
    h_dec = np.asarray(h_dec, np.float32)
    c_dec = np.asarray(c_dec, np.float32)
    encoder_output = np.asarray(encoder_output, np.float32)
    x_padding_masks = np.asarray(x_padding_masks, np.float32)
    coverage_vector = np.asarray(coverage_vector, np.float32)
    Wh_w = np.asarray(Wh_w, np.float32)
    Ws_w = np.asarray(Ws_w, np.float32)
    Ws_b = np.asarray(Ws_b, np.float32)
    v_w = np.asarray(v_w, np.float32)
    wc_w = np.asarray(wc_w, np.float32)

    whT = np.ascontiguousarray(Wh_w.T).astype(bf16)
    wsT = np.ascontiguousarray(Ws_w.T).astype(bf16)
    wsb = np.ascontiguousarray(Ws_b.reshape(NT, 128).T)
    vT = np.ascontiguousarray(v_w.reshape(NT, 128).T)
    wc = wc_w.reshape(1, D).astype(bf16)
    ident = np.eye(128, dtype=np.float32)

    s_t = np.concatenate([h_dec[0], c_dec[0]], axis=1)  # (B, D)

    in_maps = []
    for c in range(NCORES):
        sl = slice(c * BL, (c + 1) * BL)
        encTc = np.ascontiguousarray(
            encoder_output[sl].transpose(0, 2, 1)).astype(bf16)
        stTc = np.ascontiguousarray(
            s_t[sl].T.reshape(NT, 128, BL).transpose(1, 0, 2).reshape(128, NT * BL)
        ).astype(bf16)
        covfc = coverage_vector[sl].reshape(1, BL * S)
        in_maps.append({
            "encT": encTc,
            "whT": whT,
            "wsT": wsT,
            "stT": stTc,
            "wsb": wsb,
            "vT": vT,
            "wc": wc,
            "covb": covfc.astype(bf16),
            "covf": np.ascontiguousarray(covfc),
            "maskf": np.ascontiguousarray(
                x_padding_masks[sl].reshape(1, BL * S)),
            "ident": ident,
        })
    return in_maps


def kernel(**inputs):
    nc = get_nc()
    in_maps = make_in_maps(**inputs)
    # the axon-tunneled device occasionally reports a transient
    # NRT_EXEC_UNIT_UNRECOVERABLE / INTERNAL error on the first-ever
    # execution and recovers on retry — retry a few times before giving up
    last_err = None
    for attempt in range(4):
        try:
            res = run_bass_kernel_spmd(nc, in_maps,
                                       core_ids=list(range(NCORES)),
                                       trace=False).results
            break
        except Exception as e:  # noqa: BLE001
            last_err = e
            import time as _time
            _time.sleep(2.0)
    else:
        raise last_err
    ctx = np.concatenate([res[c]["ctx"] for c in range(NCORES)], axis=0)
    aw = np.concatenate([res[c]["aw"] for c in range(NCORES)], axis=0)
    ncov = np.concatenate([res[c]["ncov"] for c in range(NCORES)], axis=0)
    return ctx, aw, ncov


# revision 27
# speedup vs baseline: 1.1924x; 1.0863x over previous
"""Trainium2 Bass kernel for the coverage-attention module.

Self-contained: hardcodes shapes B=128, S=512, D=1024, 8 NeuronCores,
data-parallel over batch (16 rows per core).

Per-core dataflow (all matmuls bf16 operands, fp32 PSUM accumulation):
  att.T[e_tile, s]  = sum_dt Wh.T[dt, e_tile].T @ enc.T[dt, s]   (PE)
                      + wc[e_tile] (x) cov[s]                    (K=1 aug MM)
  tanh via ACT with per-partition bias = dec_feat.T[:, b]        (ACT)
  vsum[e, s]        = sum_et v[e] * tanh[e_tile, s], fused
                      multiply-add chain (scalar_tensor_tensor)  (DVE)
  score[1, s]       = ones.T @ vsum  (ONE M=1 matmul per batch;
                      the last two batches keep per-e_tile PE
                      score matmuls so the serial tail never
                      waits on the DVE chain)                    (PE)
  softmax on a single partition row; the reference's
  softmax->mask->renormalize collapses to e*mask/sum(e*mask)     (DVE)
  aw broadcast to 128 partitions via ones-outer-product matmul   (PE)
  context.T[dt, b]  = reduce_s(enc.T[dt, s] * aw_bcast)          (DVE)
  context transposed back via PE transpose at the end.

Startup is DMA-starved (~6 MB of weights + first enc tiles), so the
first two batches run att-only with PSUM evicted to SBUF, dec_feat is
computed after them (so Ws.T loads after enc1/enc2), and their tanh /
score / softmax are deferred until dec_feat lands. The last batch uses a
PE outer-product broadcast instead of the DMA broadcast to shorten the
serial tail.
"""

from contextlib import ExitStack

import numpy as np
import ml_dtypes

from concourse import bacc, tile, mybir
from concourse.bass_utils import run_bass_kernel_spmd

B, S, H = 128, 512, 512
D = 2 * H          # 1024
NCORES = 8
BL = B // NCORES   # 16 batch rows per core
NT = D // 128      # 8 tiles of 128 along D

BF = mybir.dt.bfloat16
F32 = mybir.dt.float32
bf16 = ml_dtypes.bfloat16

_CACHE = {}


def _build(repeat=1):
    nc = bacc.Bacc("TRN2", target_bir_lowering=False, debug=False,
                   num_devices=NCORES)

    encT = nc.dram_tensor("encT", [BL, D, S], BF, kind="ExternalInput").ap()
    whT = nc.dram_tensor("whT", [D, D], BF, kind="ExternalInput").ap()
    wsT = nc.dram_tensor("wsT", [D, D], BF, kind="ExternalInput").ap()
    stT = nc.dram_tensor("stT", [128, NT * BL], BF, kind="ExternalInput").ap()
    wsb = nc.dram_tensor("wsb", [128, NT], F32, kind="ExternalInput").ap()
    vT = nc.dram_tensor("vT", [128, NT], F32, kind="ExternalInput").ap()
    wc = nc.dram_tensor("wc", [1, D], BF, kind="ExternalInput").ap()
    covb = nc.dram_tensor("covb", [1, BL * S], BF, kind="ExternalInput").ap()
    covf = nc.dram_tensor("covf", [1, BL * S], F32, kind="ExternalInput").ap()
    maskf = nc.dram_tensor("maskf", [1, BL * S], F32, kind="ExternalInput").ap()
    ident = nc.dram_tensor("ident", [128, 128], F32, kind="ExternalInput").ap()

    ctx_o = nc.dram_tensor("ctx", [BL, D], F32, kind="ExternalOutput").ap()
    aw_o = nc.dram_tensor("aw", [BL, S], F32, kind="ExternalOutput").ap()
    ncov_o = nc.dram_tensor("ncov", [BL, S], F32, kind="ExternalOutput").ap()

    AF = mybir.ActivationFunctionType
    AX = mybir.AxisListType
    ALU = mybir.AluOpType

    with tile.TileContext(nc) as tc, ExitStack() as es:
        cp = es.enter_context(tc.tile_pool(name="const", bufs=1))

        # --- constants, in the order the PE will need them ---
        wc_sb = cp.tile([1, D], BF, tag="wc")
        nc.sync.dma_start(wc_sb[:], wc[:])

        encp = es.enter_context(tc.tile_pool(name="enc", bufs=4))

        covbp = es.enter_context(tc.tile_pool(name="covbp", bufs=4))

        def emit_covrow(b):
            cvb = covbp.tile([1, S], BF, tag="cvb")
            nc.sync.dma_start(cvb[:], covb[0:1, b * S:(b + 1) * S])
            return cvb

        def emit_load(b, nq=4):
            # nq transfers per batch: fewer dma_start instructions (the SP
            # sequencer costs ~0.5us issue per DMA) while still spreading
            # across enough HWDGE queues for bandwidth
            et_sb = encp.tile([128, NT * S], BF, tag="enc")
            tpq = NT // nq
            for q in range(nq):
                src3 = encT[b, q * tpq * 128:(q + 1) * tpq * 128, :].rearrange(
                    "(t p) s -> p t s", p=128)
                dst3 = et_sb[:, q * tpq * S:(q + 1) * tpq * S].rearrange(
                    "p (t s) -> p t s", t=tpq)
                nc.sync.dma_start(dst3, src3)
            return et_sb

        stT_sb = cp.tile([128, NT * BL], BF, tag="stT")
        nc.sync.dma_start(stT_sb[:], stT[:])
        cv0 = emit_covrow(0)
        cv1 = emit_covrow(1)
        cv2 = emit_covrow(2)

        # DMA order = consumption order: Wh.T and enc0 interleaved (b=0 att),
        # then enc1/enc2 prefetch, then Ws.T (dec runs after b1), then the
        # small constants. tanh/score for b0/b1 are deferred until dec is
        # ready, so Ws.T is not needed early.
        wht_sb = []
        enc0_sb = encp.tile([128, NT * S], BF, tag="enc")
        for t in range(NT):
            w1 = cp.tile([128, D], BF, tag=f"wht{t}", name=f"wht{t}")
            nc.sync.dma_start(w1[:], whT[t * 128:(t + 1) * 128, :])
            wht_sb.append(w1)
            nc.sync.dma_start(enc0_sb[:, t * S:(t + 1) * S],
                              encT[0, t * 128:(t + 1) * 128, :])

        enc1_sb = emit_load(1)
        enc2_sb = emit_load(2)
        wst_sb = []
        for t in range(NT):
            w2 = cp.tile([128, D], BF, tag=f"wst{t}", name=f"wst{t}")
            nc.sync.dma_start(w2[:], wsT[t * 128:(t + 1) * 128, :])
            wst_sb.append(w2)
        wsb_sb = cp.tile([128, NT], F32, tag="wsb")
        nc.sync.dma_start(wsb_sb[:], wsb[:])
        vT_sb = cp.tile([128, NT], F32, tag="vT")
        nc.sync.dma_start(vT_sb[:], vT[:])
        vTb_sb = cp.tile([128, NT], BF, tag="vTb")
        nc.vector.tensor_copy(vTb_sb[:], vT_sb[:])
        id_sb = cp.tile([128, 128], F32, tag="ident")
        nc.sync.dma_start(id_sb[:], ident[:])
        ones_sb = cp.tile([1, 128], BF, tag="ones")
        nc.vector.memset(ones_sb[:], 1.0)
        onec_sb = cp.tile([128, 1], BF, tag="onec")
        nc.vector.memset(onec_sb[:], 1.0)

        dec_sb = [cp.tile([128, BL], F32, tag=f"dec{t}", name=f"dec{t}")
                  for t in range(NT)]
        ctx_cols = [cp.tile([128, BL], F32, tag=f"ctxc{t}", name=f"ctxc{t}")
                    for t in range(NT)]
        ctx_out = cp.tile([BL, D], F32, tag="ctxout")

        tanhp = es.enter_context(tc.tile_pool(name="tanh", bufs=12))
        vsump = es.enter_context(tc.tile_pool(name="vsum", bufs=12))
        b0attp = es.enter_context(tc.tile_pool(name="b0att", bufs=NT))
        attp = es.enter_context(tc.tile_pool(name="attps", bufs=5, space="PSUM"))
        scorep = es.enter_context(tc.tile_pool(name="scoreps", bufs=2, space="PSUM"))
        trp = es.enter_context(tc.tile_pool(name="trps", bufs=1, space="PSUM"))
        smp = es.enter_context(tc.tile_pool(name="sm", bufs=2))
        awbcp = es.enter_context(tc.tile_pool(name="awbc", bufs=2))
        awdp = es.enter_context(tc.tile_pool(name="awd", bufs=2, space="DRAM"))
        tmpp = es.enter_context(tc.tile_pool(name="ctmp", bufs=2))

        def emit_att_only(b, et_sb, cvb):
            # att groups with PSUM evicted to SBUF; tanh/score deferred
            # (used for b=0/1 while dec_feat is not yet available)
            att_sb = []
            for et in range(NT):
                ap_ = attp.tile([128, S], F32, tag="att")
                nc.tensor.matmul(
                    ap_[:],
                    wc_sb[0:1, et * 128:(et + 1) * 128],
                    cvb[0:1, :],
                    start=True, stop=False,
                )
                for dt in range(NT):
                    nc.tensor.matmul(
                        ap_[:],
                        wht_sb[dt][:, et * 128:(et + 1) * 128],
                        et_sb[:, dt * S:(dt + 1) * S],
                        start=False, stop=(dt == NT - 1),
                    )
                sb = b0attp.tile([128, S], F32, tag=f"b0att{b}",
                                 name=f"b0att{b}_{et}")
                nc.scalar.copy(sb[:], ap_[:])
                att_sb.append(sb)
            return att_sb

        def emit_dec():
            # dec_feat.T columns (et, b) share ONE PSUM bank; only the very
            # first MM carries start=True — later first-writes to untouched
            # bytes reset via the per-element has_written semantics.
            dp = scorep.tile([128, NT * BL], F32, tag="score",
                             name="decps")
            for dt in range(NT):
                for et in range(NT):
                    nc.tensor.matmul(
                        dp[:, et * BL:(et + 1) * BL],
                        wst_sb[dt][:, et * 128:(et + 1) * 128],
                        stT_sb[:, dt * BL:(dt + 1) * BL],
                        start=(dt == 0 and et == 0), stop=(dt == NT - 1),
                        skip_group_check=True,
                    )
            for et in range(NT):
                nc.scalar.activation(dec_sb[et][:],
                                     dp[:, et * BL:(et + 1) * BL],
                                     AF.Identity, bias=wsb_sb[:, et:et + 1])

        def emit_vscore_step(et, b, tanh_src, chain):
            th = tanhp.tile([128, S], BF, tag="tanh")
            nc.scalar.activation(th[:], tanh_src[:], AF.Tanh,
                                 bias=dec_sb[et][:, b:b + 1])
            nxt = vsump.tile([128, S], BF, tag="vsum")
            if chain is None:
                nc.vector.tensor_scalar_mul(nxt[:], th[:],
                                            vT_sb[:, et:et + 1])
            else:
                # fused (th * v) + chain in one DVE pass
                nc.vector.scalar_tensor_tensor(
                    nxt[:], th[:], vT_sb[:, et:et + 1], chain[:],
                    op0=ALU.mult, op1=ALU.add)
            return nxt

        def emit_deferred_score(b, att_sb):
            # b0/b1 scores land in the DVE-heavy prologue region — use PE
            # M=1 score matmuls there instead (the PE has slack while the
            # enc/weight DMAs stream in)
            sc = scorep.tile([1, S], F32, tag="score", name=f"scd{b}")
            prev = None
            for et in range(NT):
                th = tanhp.tile([128, S], BF, tag="tanh")
                nc.scalar.activation(th[:], att_sb[et][:], AF.Tanh,
                                     bias=dec_sb[et][:, b:b + 1])
                if prev is not None:
                    pet, pth = prev
                    nc.tensor.matmul(sc[:], vTb_sb[:, pet:pet + 1], pth[:],
                                     start=(pet == 0), stop=False)
                prev = (et, th)
            pet, pth = prev
            nc.tensor.matmul(sc[:], vTb_sb[:, pet:pet + 1], pth[:],
                             start=False, stop=True)
            return sc

        def emit_att_score(b, et_sb, cvb, pe_score=False):
            chain = None
            sc = (scorep.tile([1, S], F32, tag="score", name="scpe")
                  if pe_score else None)
            prev = None
            for et in range(NT):
                ap_ = attp.tile([128, S], F32, tag="att")
                nc.tensor.matmul(
                    ap_[:],
                    wc_sb[0:1, et * 128:(et + 1) * 128],
                    cvb[0:1, :],
                    start=True, stop=False,
                )
                for dt in range(NT):
                    nc.tensor.matmul(
                        ap_[:],
                        wht_sb[dt][:, et * 128:(et + 1) * 128],
                        et_sb[:, dt * S:(dt + 1) * S],
                        start=False, stop=(dt == NT - 1),
                    )
                if pe_score:
                    th = tanhp.tile([128, S], BF, tag="tanh")
                    nc.scalar.activation(th[:], ap_[:], AF.Tanh,
                                         bias=dec_sb[et][:, b:b + 1])
                    if prev is not None:
                        pet, pth = prev
                        nc.tensor.matmul(sc[:], vTb_sb[:, pet:pet + 1],
                                         pth[:], start=(pet == 0), stop=False)
                    prev = (et, th)
                else:
                    chain = emit_vscore_step(et, b, ap_, chain)
            if pe_score:
                pet, pth = prev
                nc.tensor.matmul(sc[:], vTb_sb[:, pet:pet + 1], pth[:],
                                 start=False, stop=True)
                return sc
            return chain

        def emit_post(b, et_sb, vsum, last=False):
            from concourse.bass import MemorySpace
            if vsum.space == MemorySpace.PSUM:
                sc = vsum  # PE-score path already produced score in PSUM
            else:
                # score[1, s] = partition-sum of vsum via a single
                # ones-column matmul (M=1, one N=512 pass per batch)
                sc = scorep.tile([1, S], F32, tag="score")
                nc.tensor.matmul(sc[:], onec_sb[:], vsum[:], start=True,
                                 stop=True)
            exp_sb = smp.tile([1, S], F32, tag="exp")
            nc.scalar.activation(exp_sb[:], sc[:], AF.Exp)
            mrow = smp.tile([1, S], F32, tag="mrow")
            nc.sync.dma_start(mrow[:], maskf[0:1, b * S:(b + 1) * S])
            em = smp.tile([1, S], F32, tag="emask")
            nc.vector.tensor_mul(em[:], exp_sb[:], mrow[:])
            dn = smp.tile([1, 1], F32, tag="dn")
            nc.vector.reduce_sum(dn[:], em[:], axis=AX.X)
            rd = smp.tile([1, 1], F32, tag="rd")
            nc.vector.reciprocal(rd[:], dn[:])
            awr = smp.tile([1, S], F32, tag="awr")
            nc.vector.tensor_scalar_mul(awr[:], em[:], rd[:])
            cfrow = smp.tile([1, S], F32, tag="cfrow")
            nc.sync.dma_start(cfrow[:], covf[0:1, b * S:(b + 1) * S])
            ncv = smp.tile([1, S], F32, tag="ncv")
            nc.vector.tensor_add(ncv[:], awr[:], cfrow[:])
            nc.sync.dma_start(aw_o[b:b + 1, :], awr[:])
            nc.sync.dma_start(ncov_o[b:b + 1, :], ncv[:])
            awb = smp.tile([1, S], BF, tag="awb")
            nc.vector.tensor_copy(awb[:], awr[:])
            awbc = awbcp.tile([128, S], BF, tag="awbc")
            if last:
                # final batch: PE outer-product broadcast avoids two DMA
                # latencies on the kernel's serial tail
                bc = attp.tile([128, S], F32, tag="att", name="bclast")
                nc.tensor.matmul(bc[:], ones_sb[:], awb[:], start=True,
                                 stop=True)
                nc.scalar.copy(awbc[:], bc[:])
            else:
                awd = awdp.tile([1, S], BF, tag="awd")
                nc.sync.dma_start(awd[:], awb[:])
                nc.sync.dma_start(awbc[:], awd[:].to_broadcast([128, S]))
            # note: tensor_tensor_reduce wedges the device here (HW-only
            # failure; CoreSim accepts it) — use separate mul + reduce.
            for dt in range(NT):
                tmp = tmpp.tile([128, S], BF, tag="tmp")
                nc.vector.tensor_mul(tmp[:], et_sb[:, dt * S:(dt + 1) * S],
                                     awbc[:])
                nc.vector.reduce_sum(ctx_cols[dt][:, b:b + 1], tmp[:],
                                     axis=AX.X)

        for _r in range(repeat):
            if _r == 0:
                att0 = emit_att_only(0, enc0_sb, cv0)
                att1 = emit_att_only(1, enc1_sb, cv1)
                emit_dec()
                sc2 = emit_att_score(2, enc2_sb, cv2)
                sc0 = emit_deferred_score(0, att0)
                sc1 = emit_deferred_score(1, att1)
                et3_sb = emit_load(3)
                sc3 = emit_att_score(3, et3_sb, emit_covrow(3))
                emit_post(0, enc0_sb, sc0)
                emit_post(1, enc1_sb, sc1)
                prev2 = (2, enc2_sb, sc2)
                prev_state = (3, et3_sb, sc3)
                start_b = 4
            else:
                prev2 = None
                prev_state = None
                start_b = 0
            for b in range(start_b, BL):
                et_sb = emit_load(b)
                sc = emit_att_score(b, et_sb, emit_covrow(b),
                                    pe_score=(b >= BL - 2))
                if prev2 is not None:
                    emit_post(*prev2)
                    prev2 = None
                if prev_state is not None:
                    emit_post(*prev_state)
                prev_state = (b, et_sb, sc)
            emit_post(*prev_state, last=True)

            for dt in range(NT):
                tr = trp.tile([BL, 128], F32, tag="tr")
                nc.tensor.transpose(tr[:], ctx_cols[dt][:], id_sb[:])
                nc.scalar.copy(ctx_out[:, dt * 128:(dt + 1) * 128], tr[:])
            nc.sync.dma_start(ctx_o[:, :], ctx_out[:])

    nc.compile()
    return nc


def get_nc(repeat=1):
    key = ("nc", repeat)
    if key not in _CACHE:
        _CACHE[key] = _build(repeat)
    return _CACHE[key]


def make_in_maps(h_dec, c_dec, encoder_output, x_padding_masks, coverage_vector,
                 Wh_w, Ws_w, Ws_b, v_w, wc_w):
    h_dec = np.asarray(h_dec, np.float32)
    c_dec = np.asarray(c_dec, np.float32)
    encoder_output = np.asarray(encoder_output, np.float32)
    x_padding_masks = np.asarray(x_padding_masks, np.float32)
    coverage_vector = np.asarray(coverage_vector, np.float32)
    Wh_w = np.asarray(Wh_w, np.float32)
    Ws_w = np.asarray(Ws_w, np.float32)
    Ws_b = np.asarray(Ws_b, np.float32)
    v_w = np.asarray(v_w, np.float32)
    wc_w = np.asarray(wc_w, np.float32)

    whT = np.ascontiguousarray(Wh_w.T).astype(bf16)
    wsT = np.ascontiguousarray(Ws_w.T).astype(bf16)
    wsb = np.ascontiguousarray(Ws_b.reshape(NT, 128).T)
    vT = np.ascontiguousarray(v_w.reshape(NT, 128).T)
    wc = wc_w.reshape(1, D).astype(bf16)
    ident = np.eye(128, dtype=np.float32)

    s_t = np.concatenate([h_dec[0], c_dec[0]], axis=1)  # (B, D)

    in_maps = []
    for c in range(NCORES):
        sl = slice(c * BL, (c + 1) * BL)
        encTc = np.ascontiguousarray(
            encoder_output[sl].transpose(0, 2, 1)).astype(bf16)
        stTc = np.ascontiguousarray(
            s_t[sl].T.reshape(NT, 128, BL).transpose(1, 0, 2).reshape(128, NT * BL)
        ).astype(bf16)
        covfc = coverage_vector[sl].reshape(1, BL * S)
        in_maps.append({
            "encT": encTc,
            "whT": whT,
            "wsT": wsT,
            "stT": stTc,
            "wsb": wsb,
            "vT": vT,
            "wc": wc,
            "covb": covfc.astype(bf16),
            "covf": np.ascontiguousarray(covfc),
            "maskf": np.ascontiguousarray(
                x_padding_masks[sl].reshape(1, BL * S)),
            "ident": ident,
        })
    return in_maps


def kernel(**inputs):
    nc = get_nc()
    in_maps = make_in_maps(**inputs)
    # the axon-tunneled device occasionally reports a transient
    # NRT_EXEC_UNIT_UNRECOVERABLE / INTERNAL error on the first-ever
    # execution and recovers on retry — retry a few times before giving up
    last_err = None
    for attempt in range(4):
        try:
            res = run_bass_kernel_spmd(nc, in_maps,
                                       core_ids=list(range(NCORES)),
                                       trace=False).results
            break
        except Exception as e:  # noqa: BLE001
            last_err = e
            import time as _time
            _time.sleep(2.0)
    else:
        raise last_err
    ctx = np.concatenate([res[c]["ctx"] for c in range(NCORES)], axis=0)
    aw = np.concatenate([res[c]["aw"] for c in range(NCORES)], axis=0)
    ncov = np.concatenate([res[c]["ncov"] for c in range(NCORES)], axis=0)
    return ctx, aw, ncov
